# revision 6
# baseline (speedup 1.0000x reference)
"""Sliding-window attention block (B=4, S=2048, E=1024, H=16, D=64,
window_left=512, window_right=0) on 8 Trainium2 NeuronCores.

v2: fp16/bf16 dataflow. Sharding as v1: core c = (batch c//2, head group c%2).
Score path (x, Wqk, qkT, masks) in fp16; exp/V/attnT/Wout/outT in bf16
(exp needs bf16 range: unnormalized sums reach ~1e9). PSUM stays f32.
Edge key-blocks (kb0 window edge, kb5 causal edge) restricted to their live
128-query half: score MM N=128, mask MM N=128, AV MM N=128. exp stays
bank-wide (overhead-bound). The V tiles carry 64 replicated valid-ones
columns so AV matmuls land denominators on partitions 64:128 -- the
normalize is then reciprocal + fused multiply-evict with no gpsimd
broadcast. Host sums the two head-group partials (bf16->f32).
"""

import numpy as np

B, S, E, H, D = 4, 2048, 1024, 16, 64
NCORES = 8
HPC = H // 2          # heads per core
WIN = 512             # window_left (window_right = 0)
NEG = -30000.0
NQ = 256              # query stripe width
NST = S // NQ         # stripes
SCALE = 1.0 / np.sqrt(np.float32(D))

_cache = {}


def _build_program(repeat=1, ablate=()):
    from contextlib import ExitStack

    import concourse.bass as bass  # noqa: F401
    import concourse.mybir as mybir
    import concourse.tile as tile
    from concourse import bacc

    dt = mybir.dt
    f32, f16, bf16 = dt.float32, dt.float16, dt.bfloat16
    AF = mybir.ActivationFunctionType
    mult = mybir.AluOpType.mult

    nc = bacc.Bacc("TRN2", target_bir_lowering=False, debug=False,
                   num_devices=NCORES)

    xT = nc.dram_tensor("xT", [E, S], f16, kind="ExternalInput")
    wqk = nc.dram_tensor("wqk", [E, 2 * HPC * D], f16, kind="ExternalInput")
    wv = nc.dram_tensor("wv", [E, HPC * D], f16, kind="ExternalInput")
    wo = nc.dram_tensor("wo", [HPC * D, E], bf16, kind="ExternalInput")
    vmask = nc.dram_tensor("vmask", [128, 16], f32, kind="ExternalInput")
    # masks: cols 0:512 = maskAB, 512:1024 = maskCD, 1024:1152 = identity
    masks = nc.dram_tensor("masks", [128, 1152], f16, kind="ExternalInput")
    outT = nc.dram_tensor("outT", [E, S], bf16, kind="ExternalOutput")

    with tile.TileContext(nc) as tc, \
         nc.allow_low_precision(reason="2e-2 tolerance; fp16/bf16 validated"):
      for _rep in range(repeat):
       with ExitStack() as ctx:
        persist = ctx.enter_context(tc.tile_pool(name="persist", bufs=1))

        qkT = [persist.tile([128, S], f16, name=f"qkT{i}", tag=f"qkT{i}")
               for i in range(8)]
        vsb = [persist.tile([128, HPC, 2 * D], bf16, name=f"v{t}",
                            tag=f"v{t}") for t in range(16)]
        msk = persist.tile([128, 1152], f16, tag="msk")
        maskAB = msk[:, 0:512]
        maskCD = msk[:, 512:1024]
        ident = msk[:, 1024:1152]
        vmsb = persist.tile([128, 16], f32, tag="vmsb")
        onesT = persist.tile([128, HPC, D], bf16, tag="onesT")
        nc.vector.memset(onesT[:, :, :], 1.0)

        # ---- phase 1+2: qk projection (feature-major) + V (seq-major) ----
        with tc.tile_pool(name="wgt12", bufs=1) as wpool, \
             tc.tile_pool(name="xc", bufs=3) as xpool, \
             tc.tile_pool(name="qkps", bufs=2 if "qk2" in ablate else 4,
                          space="PSUM") as qkps, \
             tc.tile_pool(name="vps", bufs=2 if "qk2" in ablate else 4,
                          space="PSUM") as vps:
            xcs = {}

            def load_chunk(nb):
                tiles = [xpool.tile([128, 512], f16, name=f"xc{k}",
                                    tag=f"xc{k}") for k in range(8)]
                for k in range(8):
                    nc.sync.dma_start(
                        out=tiles[k],
                        in_=xT[k * 128:(k + 1) * 128,
                               nb * 512:(nb + 1) * 512])
                xcs[nb] = tiles

            load_chunk(0)
            wqk_sb = [wpool.tile([128, 2 * HPC * D], f16, name=f"wqk{k}",
                                 tag=f"wqk{k}") for k in range(8)]
            for k in range(8):
                if "nowdma" not in ablate:
                    nc.sync.dma_start(out=wqk_sb[k],
                                      in_=wqk[k * 128:(k + 1) * 128, :])
                else:
                    nc.vector.memset(wqk_sb[k][:, 0:8], 0.125)
            wv_sb = [wpool.tile([128, HPC * D], f16, name=f"wv{k}",
                                tag=f"wv{k}") for k in range(8)]
            for k in range(8):
                if "nowdma" not in ablate:
                    nc.sync.dma_start(out=wv_sb[k],
                                      in_=wv[k * 128:(k + 1) * 128, :])
                else:
                    nc.vector.memset(wv_sb[k][:, 0:8], 0.125)
            nc.sync.dma_start(out=msk, in_=masks[:, :])
            nc.sync.dma_start(out=vmsb, in_=vmask[:, :])

            for nb in range(4):
                if "noxdma" in ablate:
                    if 0 not in xcs:
                        load_chunk(0)
                    xc = xcs[0]
                else:
                    if nb not in xcs:
                        load_chunk(nb)
                    xc = xcs.pop(nb)
                    if nb + 1 < 4 and (nb + 1) not in xcs:
                        load_chunk(nb + 1)
                for mb in range(8):
                    ps = qkps.tile([128, 512], f32, tag="qk")
                    for k in range(8):
                        nc.tensor.matmul(
                            ps[:, :],
                            lhsT=wqk_sb[k][:, mb * 128:(mb + 1) * 128],
                            rhs=xc[k][:, :],
                            start=(k == 0), stop=(k == 7))
                    dst = qkT[mb][:, nb * 512:(nb + 1) * 512]
                    if mb % 2 == 0:
                        nc.scalar.copy(dst, ps[:, :])
                    else:
                        nc.vector.tensor_copy(dst, ps[:, :])
                for t4 in range(4):
                    t = nb * 4 + t4
                    ps = vps.tile([128, 512], f32, tag="v")
                    for k in range(8):
                        nc.tensor.matmul(
                            ps[:, :],
                            lhsT=xc[k][:, t4 * 128:(t4 + 1) * 128],
                            rhs=wv_sb[k][:, :],
                            start=(k == 0), stop=(k == 7))
                    nc.vector.tensor_scalar(
                        out=vsb[t][:, :, 0:D],
                        in0=ps.rearrange("p (h d) -> p h d", h=HPC),
                        scalar1=vmsb[:, t:t + 1],
                        scalar2=None,
                        op0=mult)
                    nc.vector.tensor_scalar(
                        out=vsb[t][:, :, D:2 * D],
                        in0=onesT[:, :, :],
                        scalar1=vmsb[:, t:t + 1],
                        scalar2=None,
                        op0=mult)

        # ---- phase 3: banded attention + interleaved output projection ----
        apool = ctx.enter_context(tc.tile_pool(name="attn", bufs=1))
        attnT = [apool.tile([128, S], bf16, name=f"attnT{i}", tag=f"attnT{i}")
                 for i in range(4)]
        wo_sb = [apool.tile([128, E], bf16, name=f"wo{c}", tag=f"wo{c}")
                 for c in range(4)]
        for c in range(4):
            nc.sync.dma_start(out=wo_sb[c], in_=wo[c * 128:(c + 1) * 128, :])
        nsp = 5 if "sp5" in ablate else 4
        with tc.tile_pool(name="scps", bufs=nsp, space="PSUM") as spool, \
             tc.tile_pool(name="ops", bufs=2, space="PSUM") as opool, \
             tc.tile_pool(name="oprj", bufs=1 if nsp == 5 else 2,
                          space="PSUM") as ppool, \
             tc.tile_pool(name="expT", bufs=8) as epool, \
             tc.tile_pool(name="ob", bufs=3) as obpool, \
             tc.tile_pool(name="rc", bufs=4) as rpool:
            for s in range(NST):
                if s == 0:
                    kbs = [4, 5]
                elif s == 1:
                    kbs = [2, 3, 4, 5]
                else:
                    kbs = [0, 1, 2, 3, 4, 5]
                nkb = len(kbs)
                base_kt = 2 * s - 4
                # per key-block: (live query col range, in a masked bank?)
                # kb0 lives on cols 0:128, kb5 on 128:256 (window/causal
                # edges); masks are applied bank-wide (maskAB / maskCD)
                kbcols = {0: (0, 128), 1: (0, 256), 2: (0, 256),
                          3: (0, 256), 4: (0, 256), 5: (128, 256)}
                if "uni256" in ablate:
                    kbcols = {k: (0, 256) for k in range(6)}
                banks = [kbs[b0:b0 + 2] for b0 in range(0, nkb, 2)]
                bank_mask = {}
                for bi, bkbs in enumerate(banks):
                    if "mask" in ablate:
                        break
                    if bkbs[0] == 0:
                        bank_mask[bi] = maskAB
                    elif bkbs[0] == 4:
                        bank_mask[bi] = maskCD
                if "noatt" in ablate:
                    if s == 0:
                        for i in range(4):
                            nc.vector.memset(attnT[i][:, 0:8], 0.125)
                    banks = []
                for hp in range(4 if "noatt" not in ablate else 0):
                  g = hp  # both heads of the pair live in qkT[g]/attnT[g]
                  h0, h1 = 2 * hp, 2 * hp + 1
                  exs = []  # per bank: [exb_h0, exb_h1]
                  for bi, bkbs in enumerate(banks):
                    scb = [spool.tile([128, 2, NQ], f32, name=f"sc{hh}",
                                      tag="sc") for hh in (0, 1)]
                    # interleave the two heads' score MMs: adjacent MMs sit
                    # on disjoint PE row groups (partitions 0:64 / 64:128)
                    # and run concurrently
                    mops = []  # (head_idx, out_ap, lhsT, rhs)
                    for li, kb in enumerate(bkbs):
                        c0, c1 = kbcols[kb]
                        ktile = base_kt + kb
                        for hi, po in ((0, 0), (1, 64)):
                            mops.append((
                                hi,
                                scb[hi][:, li, c0:c1],
                                qkT[4 + g][po:po + 64,
                                           ktile * 128:(ktile + 1) * 128],
                                qkT[g][po:po + 64,
                                       s * NQ + c0:s * NQ + c1]))
                    mk = bank_mask.get(bi)
                    if mk is not None:
                        for hi in (0, 1):
                            mops.append((hi, scb[hi][:, :, :], ident,
                                         mk[:, :]))
                    if "nopair" in ablate:
                        mops = ([m for m in mops if m[0] == 0]
                                + [m for m in mops if m[0] == 1])
                    seen = [0, 0]
                    tot = [sum(1 for m in mops if m[0] == 0),
                           sum(1 for m in mops if m[0] == 1)]
                    for hi, out_ap, lhsT, rhs in mops:
                        nc.tensor.matmul(out_ap, lhsT=lhsT, rhs=rhs,
                                         start=(seen[hi] == 0),
                                         stop=(seen[hi] == tot[hi] - 1))
                        seen[hi] += 1
                    exb = [epool.tile([128, 2, NQ], bf16, name=f"ex{hh}",
                                      tag="ex") for hh in (0, 1)]
                    for hi in (0, 1):
                        if "cheapexp" in ablate:
                            nc.scalar.activation(exb[hi][:, 0, :],
                                                 scb[hi][:, 0, :], AF.Exp)
                        elif "fexp" not in ablate and bkbs[0] == 0:
                            nc.scalar.activation(exb[hi][:, 0, 0:128],
                                                 scb[hi][:, 0, 0:128], AF.Exp)
                            nc.scalar.activation(exb[hi][:, 1, :],
                                                 scb[hi][:, 1, :], AF.Exp)
                        elif "fexp" not in ablate and bkbs[0] == 4:
                            nc.scalar.activation(exb[hi][:, 0, :],
                                                 scb[hi][:, 0, :], AF.Exp)
                            nc.scalar.activation(exb[hi][:, 1, 128:256],
                                                 scb[hi][:, 1, 128:256], AF.Exp)
                        else:
                            nc.scalar.activation(exb[hi][:, :, :],
                                                 scb[hi][:, :, :], AF.Exp)
                    exs.append(exb)
                  # AV for both heads into one shared PSUM bank:
                  # ot[:, hi, :] = head hi; row D = denominators
                  ot = opool.tile([128, 2, NQ], f32, tag="ot")
                  nav = 2 * nkb
                  iav = 0
                  for hi, po in ((0, 0), (1, 64)):
                    for i, kb in enumerate(kbs):
                      c0, c1 = kbcols[kb]
                      ktile = base_kt + kb
                      nc.tensor.matmul(
                          ot[:, hi, c0:c1],
                          lhsT=vsb[ktile][:, 2 * hp + hi, :],
                          rhs=exs[i // 2][hi][:, i % 2, c0:c1],
                          start=(iav == 0), stop=(iav == nav - 1))
                      iav += 1
                  if "cpnorm" in ablate:
                    st = rpool.tile([128, 2, NQ], bf16, tag="st")
                    if hp % 2 == 0:
                        nc.scalar.copy(st[:, :, :], ot[:, :, :])
                    else:
                        nc.vector.tensor_copy(st[:, :, :], ot[:, :, :])
                    rc = rpool.tile([D, 2, NQ], bf16, tag="rc")
                    nc.vector.reciprocal(rc[:, :, :], st[D:2 * D, :, :])
                    for hi, po in ((0, 0), (1, 64)):
                      dst = attnT[g][po:po + 64, s * NQ:(s + 1) * NQ]
                      nc.vector.tensor_tensor(
                          out=dst, in0=st[0:D, hi, :],
                          in1=rc[:, hi, :], op=mult)
                  elif "norm" in ablate:
                    for hi, po in ((0, 0), (1, 64)):
                      dst = attnT[g][po:po + 64, s * NQ:(s + 1) * NQ]
                      nc.vector.tensor_copy(dst, ot[0:D, hi, :])
                  else:
                    # denominators are replicated on partitions 64:128 by the
                    # ones columns of vsb; reciprocal them directly (no
                    # cross-partition broadcast needed), then fused evict
                    rc = rpool.tile([D, 2, NQ], bf16, tag="rc")
                    nc.vector.reciprocal(rc[:, :, :], ot[D:2 * D, :, :])
                    for hi, po in ((0, 0), (1, 64)):
                      dst = attnT[g][po:po + 64, s * NQ:(s + 1) * NQ]
                      nc.vector.tensor_tensor(
                          out=dst, in0=ot[0:D, hi, :],
                          in1=rc[:, hi, :], op=mult)

                # output projection for the completed stripe pair
                if s % 2 == 1 and "noop" not in ablate:
                    c0 = (s - 1) * NQ
                    for mb in range(8):
                        pp = ppool.tile([128, 2 * NQ], f32, tag="pp")
                        for cb in range(4):
                            nc.tensor.matmul(
                                pp[:, :],
                                lhsT=wo_sb[cb][:, mb * 128:(mb + 1) * 128],
                                rhs=attnT[cb][:, c0:c0 + 2 * NQ],
                                start=(cb == 0), stop=(cb == 3))
                        ob = obpool.tile([128, 2 * NQ], bf16, tag="ob")
                        if "oact" in ablate:
                            nc.scalar.copy(ob[:, :], pp[:, :])
                        elif "opool" in ablate:
                            nc.gpsimd.tensor_copy(ob[:, :], pp[:, :])
                        else:
                            nc.vector.tensor_copy(ob[:, :], pp[:, :])
                        if "noout" not in ablate:
                            eng = (nc.sync if "sdma" in ablate else
                                   (nc.gpsimd if mb % 2 == 0 else nc.sync))
                            eng.dma_start(
                                out=outT[mb * 128:(mb + 1) * 128,
                                         c0:c0 + 2 * NQ],
                                in_=ob[:, :])

    nc.compile()
    return nc


def _prep_inputs(x_padded, Wqkv, Wout, seq_lengths, mask_dtype=None):
    """Per-core input maps (mask_dtype ignored; kept for test.py compat)."""
    import ml_dtypes
    bf16 = ml_dtypes.bfloat16

    Wq = Wqkv[0:E]
    Wk = Wqkv[E:2 * E]
    Wv = Wqkv[2 * E:3 * E]

    p = np.arange(128)[:, None]
    f = np.arange(NQ)[None, :]
    m_a = np.where(f <= p, 0.0, NEG)
    m_b = np.where(f <= p + 128, 0.0, NEG)
    m_c = np.where(f >= p, 0.0, NEG)
    m_d = np.where(f >= p + 128, 0.0, NEG)
    ident = np.eye(128)
    masks = np.concatenate([m_a, m_b, m_c, m_d, ident],
                           axis=1).astype(np.float16)

    in_maps = []
    for c in range(NCORES):
        b, g = divmod(c, 2)
        hs = np.arange(g * HPC, (g + 1) * HPC)
        rows = (hs[:, None] * D + np.arange(D)[None, :]).reshape(-1)
        wqk_c = np.concatenate([Wq[rows] * SCALE, Wk[rows]], axis=0)
        valid = (np.arange(S) < seq_lengths[b]).astype(np.float32)
        in_maps.append({
            "xT": np.ascontiguousarray(x_padded[b].T).astype(np.float16),
            "wqk": np.ascontiguousarray(wqk_c.T).astype(np.float16),
            "wv": np.ascontiguousarray(Wv[rows].T).astype(np.float16),
            "wo": np.ascontiguousarray(Wout[:, rows].T).astype(bf16),
            "vmask": np.ascontiguousarray(valid.reshape(16, 128).T),
            "masks": masks,
        })
    return in_maps


def _make_runner(nc):
    """Reusable jitted SPMD executor."""
    import jax
    import numpy as np
    from jax.experimental.shard_map import shard_map
    from jax.sharding import Mesh, PartitionSpec

    import concourse.mybir as mybir
    from concourse.bass2jax import (_bass_exec_p, install_neuronx_cc_hook,
                                    partition_id_tensor)

    install_neuronx_cc_hook()
    partition_name = (nc.partition_id_tensor.name
                      if nc.partition_id_tensor else None)
    in_names, out_names, out_avals, zero_outs = [], [], [], []
    for alloc in nc.m.functions[0].allocations:
        if not isinstance(alloc, mybir.MemoryLocationSet):
            continue
        name = alloc.memorylocations[0].name
        if alloc.kind == "ExternalInput":
            if name != partition_name:
                in_names.append(name)
        elif alloc.kind == "ExternalOutput":
            shape = tuple(alloc.tensor_shape)
            dtype = mybir.dt.np(alloc.dtype)
            out_names.append(name)
            out_avals.append(jax.core.ShapedArray(shape, dtype))
            zero_outs.append(np.zeros(shape, dtype))
    n_params = len(in_names)
    n_outs = len(out_avals)
    all_in_names = list(in_names) + list(out_names)
    if partition_name is not None:
        all_in_names.append(partition_name)
    donate = tuple(range(n_params, n_params + n_outs))

    def _body(*args):
        operands = list(args)
        if partition_name is not None:
            operands.append(partition_id_tensor())
        outs = _bass_exec_p.bind(
            *operands,
            out_avals=tuple(out_avals),
            in_names=tuple(all_in_names),
            out_names=tuple(out_names),
            lowering_input_output_aliases=(),
            sim_require_finite=True,
            sim_require_nnan=True,
            nc=nc,
        )
        return tuple(outs)

    devices = jax.devices()[:NCORES]
    mesh = Mesh(np.asarray(devices), ("core",))
    in_specs = (PartitionSpec("core"),) * (n_params + n_outs)
    out_specs = (PartitionSpec("core"),) * len(out_names)
    sharded = jax.jit(
        shard_map(_body, mesh=mesh, in_specs=in_specs, out_specs=out_specs,
                  check_rep=False),
        donate_argnums=donate, keep_unused=True)

    def prep(in_maps):
        concat_in = [
            np.concatenate([np.asarray(in_maps[c][nm]) for c in range(NCORES)],
                           axis=0)
            for nm in in_names]
        concat_zeros = [np.zeros((NCORES * z.shape[0], *z.shape[1:]), z.dtype)
                        for z in zero_outs]
        return concat_in, concat_zeros

    def run_prepped(concat_in, concat_zeros):
        return sharded(*concat_in, *concat_zeros)

    def run(in_maps):
        concat_in, concat_zeros = prep(in_maps)
        out_arrs = run_prepped(concat_in, concat_zeros)
        return [
            {nm: np.asarray(out_arrs[i]).reshape(NCORES, *out_avals[i].shape)[c]
             for i, nm in enumerate(out_names)}
            for c in range(NCORES)]

    run.prep = prep
    run.run_prepped = run_prepped
    run.mesh = mesh
    return run


def get_runner():
    if "runner" not in _cache:
        if "nc" not in _cache:
            _cache["nc"] = _build_program()
        _cache["runner"] = _make_runner(_cache["nc"])
    return _cache["runner"]


def kernel(x_padded, Wqkv, Wout, seq_lengths, window_left, window_right):
    assert int(window_left) == WIN and int(window_right) == 0
    x_padded = np.asarray(x_padded, dtype=np.float32)
    Wqkv = np.asarray(Wqkv, dtype=np.float32)
    Wout = np.asarray(Wout, dtype=np.float32)
    seq_lengths = np.asarray(seq_lengths, dtype=np.int32)

    run = get_runner()
    in_maps = _prep_inputs(x_padded, Wqkv, Wout, seq_lengths)
    results = run(in_maps)

    out = np.empty((B, S, E), dtype=np.float32)
    for b in range(B):
        acc = (results[2 * b]["outT"].astype(np.float32)
               + results[2 * b + 1]["outT"].astype(np.float32))
        out[b] = acc.T

    # fully-masked query rows: window [i-512, i] entirely past seq_len
    Wv = Wqkv[2 * E:3 * E]
    for b in range(B):
        sl = int(seq_lengths[b])
        if sl == 0:
            v_mean = x_padded[b].mean(axis=0) @ Wv.T
            out[b, :, :] = v_mean @ Wout.T
        elif sl + WIN < S:
            v_mean = x_padded[b].mean(axis=0) @ Wv.T
            out[b, sl + WIN:, :] = v_mean @ Wout.T
    return out
